# revision 8
# baseline (speedup 1.0000x reference)
"""Trainium2 Bass kernel for nn_CrossAttention_47502338294587.

Math: the reference cross-attention has a single KV position broadcast over
all T query positions.  Softmax over a row of identical logits is uniform,
so attention output == v for every query, and the whole module collapses to

    out[b, t, :] = (visual_features[b] @ Wv + bv) @ Wp + bp      (for all t)

independent of x / Wq / Wk.  The device computes the two projections and
broadcasts the per-batch row over its T-shard; the host only re-assembles
shards (pure layout, no arithmetic).

Sharding: data-parallel over T across the 8 cores — core i writes
out[:, i*128:(i+1)*128, :].
"""

import os
import sys

import numpy as np

for _p in ("/opt/trn_rl_repo",):
    if _p not in sys.path and os.path.isdir(_p):
        sys.path.insert(0, _p)

B, T, C = 4, 1024, 1024
N_CORES = 8
TSH = T // N_CORES  # 128, T-shard per core
KC = C // 128  # 8 contraction chunks

_BUILT = None


def build_nc():
    """Build + compile the Bass program (one NeuronCore's SPMD body)."""
    import concourse.mybir as mybir
    import concourse.tile as tile
    from concourse import bacc
    from concourse.bass import ts

    f32 = mybir.dt.float32
    nc = bacc.Bacc("TRN2", target_bir_lowering=False, debug=False)

    wv = nc.dram_tensor("wv", [C, C], f32, kind="ExternalInput")
    wp = nc.dram_tensor("wp", [C, C], f32, kind="ExternalInput")
    vft = nc.dram_tensor("vft", [C, B], f32, kind="ExternalInput")  # vf transposed
    bv = nc.dram_tensor("bv", [1, C], f32, kind="ExternalInput")
    bp = nc.dram_tensor("bp", [1, C], f32, kind="ExternalInput")
    out = nc.dram_tensor("out", [TSH, B, C], f32, kind="ExternalOutput")

    with tile.TileContext(nc) as tc:
        with tc.tile_pool(name="sb", bufs=1) as sb:
            # ---- SBUF tiles -------------------------------------------------
            wv_t = [sb.tile([128, C], f32, name=f"wv{k}", tag=f"wv{k}") for k in range(KC)]
            wp_t = [sb.tile([128, C], f32, name=f"wp{k}", tag=f"wp{k}") for k in range(KC)]
            vft_t = sb.tile([128, KC, B], f32, tag="vft")
            bv_t = sb.tile([1, C], f32, tag="bv")
            bp_t = sb.tile([1, C], f32, tag="bp")
            ones_b = sb.tile([1, B], f32, tag="ones_b")
            # sel[k, b*128 + p] = (k == b): stationary selector that both picks
            # batch row b and broadcasts it across all 128 output partitions.
            sel_t = sb.tile([B, B * 128], f32, tag="sel")
            vvt_t = sb.tile([128, KC * B], f32, tag="vvt")  # vv^T, chunk k at cols [4k,4k+4)
            row_t = sb.tile([B, C], f32, tag="row")
            bc_t = sb.tile([128, B * C], f32, tag="bc")  # broadcast rows, b at cols [1024b, ...)

            nc.vector.memset(ones_b[:], 1.0)
            # sel[k, y] = 1 iff 128k <= y <= 128k+127 (block-diagonal selector)
            nc.gpsimd.memset(sel_t[:], 1.0)
            nc.gpsimd.affine_select(
                out=sel_t[:],
                in_=sel_t[:],
                compare_op=mybir.AluOpType.is_ge,
                fill=0.0,
                base=0,
                pattern=[[1, B * 128]],  # expr = y - 128*k
                channel_multiplier=-128,
            )
            nc.gpsimd.affine_select(
                out=sel_t[:],
                in_=sel_t[:],
                compare_op=mybir.AluOpType.is_ge,
                fill=0.0,
                base=127,  # expr = 127 + 128*k - y
                pattern=[[-1, B * 128]],
                channel_multiplier=128,
            )

            # ---- DMA in -----------------------------------------------------
            nc.sync.dma_start(vft_t[:], vft.rearrange("(k p) b -> p k b", p=128))
            nc.sync.dma_start(bv_t[:], bv[:, :])
            nc.sync.dma_start(bp_t[:], bp[:, :])
            for k in range(KC):
                nc.sync.dma_start(wv_t[k][:], wv[ts(k, 128), :])
            for k in range(KC):
                nc.sync.dma_start(wp_t[k][:], wp[ts(k, 128), :])

            # ---- mm1: vv^T[n, b] = sum_k Wv[k, n] * vf[b, k] + bv[n] --------
            # orientation: stationary = Wv chunk [K=128, M=128], moving = vf^T [K=128, B]
            with tc.tile_pool(name="pv", bufs=1, space="PSUM") as pv:
                psum_vvt = [pv.tile([128, B], f32, name=f"pv{n}", tag=f"pv{n}") for n in range(KC)]
                for k in range(KC):
                    for n in range(KC):
                        nc.tensor.matmul(
                            psum_vvt[n][:],
                            wv_t[k][:, ts(n, 128)],
                            vft_t[:, k, :],
                            start=(k == 0),
                            stop=False,
                        )
                # bias row: vv^T[n, b] += bv[n] * 1   (K=1 matmul)
                for n in range(KC):
                    nc.tensor.matmul(
                        psum_vvt[n][:],
                        bv_t[0:1, ts(n, 128)],
                        ones_b[0:1, :],
                        start=False,
                        stop=True,
                    )
                for n in range(KC):
                    nc.vector.tensor_copy(vvt_t[:, ts(n, B)], psum_vvt[n][:])

            # ---- mm2 + broadcast, per 512-wide half of C --------------------
            # mm2: row[b, n] = sum_k vv[b, k] * Wp[k, n] + bp[n]
            #   stationary = vv^T chunk [K=128, M=B], moving = Wp chunk [K=128, 512]
            # bcast: bc[t, n] = row[b, n] for all t (ones-column outer product)
            with (
                tc.tile_pool(name="pr", bufs=2, space="PSUM") as pr,
                tc.tile_pool(name="pb", bufs=4, space="PSUM") as pb,
            ):
                for h in range(2):
                    psum_row = pr.tile([B, 512], f32, tag="pr")
                    for k in range(KC):
                        nc.tensor.matmul(
                            psum_row[:],
                            vvt_t[:, ts(k, B)],
                            wp_t[k][:, ts(h, 512)],
                            start=(k == 0),
                            stop=False,
                        )
                    nc.tensor.matmul(
                        psum_row[:],
                        ones_b[0:1, :],
                        bp_t[0:1, ts(h, 512)],
                        start=False,
                        stop=True,
                    )
                    nc.vector.tensor_copy(row_t[0:B, ts(h, 512)], psum_row[:])

                    for b in range(B):
                        psum_bc = pb.tile([128, 512], f32, tag="pb")
                        nc.tensor.matmul(
                            psum_bc[:],
                            sel_t[0:B, ts(b, 128)],
                            row_t[0:B, ts(h, 512)],
                            start=True,
                            stop=True,
                        )
                        nc.vector.tensor_copy(
                            bc_t[:, b * C + h * 512 : b * C + h * 512 + 512],
                            psum_bc[:],
                        )

                    # out[:, :, h*512:(h+1)*512] <- bc half h
                    nc.sync.dma_start(
                        out[:, :, ts(h, 512)],
                        bc_t[:].rearrange("p (b c) -> p b c", b=B)[:, :, ts(h, 512)],
                    )

    nc.compile()
    return nc


def _get_built():
    global _BUILT
    if _BUILT is None:
        _BUILT = build_nc()
    return _BUILT


def make_in_map(inputs):
    vf = np.ascontiguousarray(np.asarray(inputs["visual_features"], np.float32))
    return {
        "wv": np.ascontiguousarray(np.asarray(inputs["Wv"], np.float32)),
        "wp": np.ascontiguousarray(np.asarray(inputs["Wp"], np.float32)),
        "vft": np.ascontiguousarray(vf.T),
        "bv": np.ascontiguousarray(np.asarray(inputs["bv"], np.float32))[None, :],
        "bp": np.ascontiguousarray(np.asarray(inputs["bp"], np.float32))[None, :],
    }


def run(inputs, trace=False, **kw):
    from concourse.bass_utils import run_bass_kernel_spmd

    nc = _get_built()
    in_map = make_in_map(inputs)
    res = run_bass_kernel_spmd(
        nc,
        [dict(in_map) for _ in range(N_CORES)],
        core_ids=list(range(N_CORES)),
        trace=trace,
        **kw,
    )
    full = np.empty((B, T, C), np.float32)
    for i, r in enumerate(res.results):
        full[:, i * TSH : (i + 1) * TSH, :] = r["out"].transpose(1, 0, 2)
    return full, res


def kernel(**inputs) -> np.ndarray:
    full, _ = run(inputs, trace=False)
    return full


# revision 11
# speedup vs baseline: 1.1836x; 1.1836x over previous
"""Trainium2 Bass kernel for nn_CrossAttention_47502338294587.

Math: the reference cross-attention has a single KV position broadcast over
all T query positions.  Softmax over a row of identical logits is uniform,
so attention output == v for every query, and the whole module collapses to

    out[b, t, :] = (visual_features[b] @ Wv + bv) @ Wp + bp      (for all t)

independent of x / Wq / Wk.  The device computes the two projections and
broadcasts the per-batch row over its T-shard; the host only re-assembles
shards (pure layout, no arithmetic).

Sharding: data-parallel over T across the 8 cores — core i writes
out[:, i*128:(i+1)*128, :].

Kernel structure (per core):
  mm1:   vv = vf @ Wv + bv        stationary vf^T (tiny ldweights), N=512 tiles
  tr:    vv^T chunks via PE transpose
  mm2:   row = vv @ Wp + bp       stationary vv^T chunks
  bcast: out[t, b, :] = row[b, :] selector-matmul replicates across partitions,
         PSUM DMA'd straight to DRAM
"""

import os
import sys

import numpy as np

for _p in ("/opt/trn_rl_repo",):
    if _p not in sys.path and os.path.isdir(_p):
        sys.path.insert(0, _p)

B, T, C = 4, 1024, 1024
N_CORES = 8
TSH = T // N_CORES  # 128, T-shard per core
KC = C // 128  # 8 contraction chunks

_BUILT = None


def build_nc():
    """Build + compile the Bass program (one NeuronCore's SPMD body)."""
    import concourse.mybir as mybir
    import concourse.tile as tile
    from concourse import bacc
    from concourse.bass import ts

    f32 = mybir.dt.float32
    nc = bacc.Bacc("TRN2", target_bir_lowering=False, debug=False)

    wv = nc.dram_tensor("wv", [C, C], f32, kind="ExternalInput")
    wp = nc.dram_tensor("wp", [C, C], f32, kind="ExternalInput")
    vft = nc.dram_tensor("vft", [C, B], f32, kind="ExternalInput")  # vf transposed
    bv = nc.dram_tensor("bv", [1, C], f32, kind="ExternalInput")
    bp = nc.dram_tensor("bp", [1, C], f32, kind="ExternalInput")
    out = nc.dram_tensor("out", [TSH, B, C], f32, kind="ExternalOutput")

    def band_select(ap, mult, width):
        """keep 1.0 inside the band 0 <= y - mult*k <= width-1, else 0."""
        nc.gpsimd.memset(ap, 1.0)
        nc.gpsimd.affine_select(
            out=ap, in_=ap, compare_op=mybir.AluOpType.is_ge, fill=0.0,
            base=0, pattern=[[1, ap.shape[-1]]], channel_multiplier=-mult,
        )
        nc.gpsimd.affine_select(
            out=ap, in_=ap, compare_op=mybir.AluOpType.is_ge, fill=0.0,
            base=width - 1, pattern=[[-1, ap.shape[-1]]], channel_multiplier=mult,
        )

    with tile.TileContext(nc) as tc:
        with tc.tile_pool(name="sb", bufs=1) as sb:
            # ---- SBUF tiles -------------------------------------------------
            wv_t = [sb.tile([128, C], f32, name=f"wv{k}", tag=f"wv{k}") for k in range(KC)]
            wp_t = [sb.tile([128, C], f32, name=f"wp{k}", tag=f"wp{k}") for k in range(KC)]
            vft_t = sb.tile([128, KC, B], f32, tag="vft")
            bv_t = sb.tile([1, C], f32, tag="bv")
            bp_t = sb.tile([1, C], f32, tag="bp")
            ones_b = sb.tile([1, B], f32, tag="ones_b")
            # sel[k, b*128 + p] = (k == b): stationary selector that both picks
            # batch row b and broadcasts it across all 128 output partitions.
            sel_t = sb.tile([B, B * 128], f32, tag="sel")
            ident_t = sb.tile([B, B], f32, tag="ident")
            vv_sb = sb.tile([B, C], f32, tag="vv_sb")
            vvt_t = sb.tile([128, KC * B], f32, tag="vvt")  # vv^T, chunk k at cols [4k,4k+4)
            row_t = sb.tile([B, C], f32, tag="row")
            bc_t = [sb.tile([128, B * 512], f32, name=f"bc{h}", tag=f"bc{h}") for h in range(2)]

            nc.vector.memset(ones_b[:], 1.0)
            band_select(sel_t[:], 128, 128)
            band_select(ident_t[:], 1, 1)

            # ---- DMA in -----------------------------------------------------
            nc.sync.dma_start(vft_t[:], vft.rearrange("(k p) b -> p k b", p=128))
            nc.sync.dma_start(bv_t[:], bv[:, :])
            nc.sync.dma_start(bp_t[:], bp[:, :])
            for k in range(KC):
                nc.sync.dma_start(wv_t[k][:], wv[ts(k, 128), :])
            for k in range(KC):
                nc.sync.dma_start(wp_t[k][:], wp[ts(k, 128), :])

            # ---- mm1: vv[b, n] = sum_k vf[b, k] Wv[k, n] + bv[n] ------------
            # stationary = vf^T chunk [K=128, M=B], moving = Wv chunk [K=128, 512]
            with tc.tile_pool(name="pv", bufs=2, space="PSUM") as pv:
                psum_vv = [pv.tile([B, 512], f32, name=f"pvv{h}", tag=f"pvv{h}") for h in range(2)]
                for k in range(KC):
                    for h in range(2):
                        nc.tensor.matmul(
                            psum_vv[h][:],
                            vft_t[:, k, :],
                            wv_t[k][:, ts(h, 512)],
                            start=(k == 0),
                            stop=False,
                        )
                for h in range(2):
                    nc.tensor.matmul(
                        psum_vv[h][:],
                        ones_b[0:1, :],
                        bv_t[0:1, ts(h, 512)],
                        start=False,
                        stop=True,
                    )
                for h in range(2):
                    nc.vector.tensor_copy(vv_sb[0:B, ts(h, 512)], psum_vv[h][:])

            # ---- transpose vv -> vv^T chunks [128, B] -----------------------
            with tc.tile_pool(name="pt", bufs=4, space="PSUM") as pt:
                for k in range(KC):
                    psum_vvt = pt.tile([128, B], f32, tag="pvt")
                    nc.tensor.transpose(
                        psum_vvt[:], vv_sb[0:B, ts(k, 128)], ident_t[0:B, 0:B]
                    )
                    nc.vector.tensor_copy(vvt_t[:, ts(k, B)], psum_vvt[:])

            # ---- mm2 + broadcast, per 512-wide half of C --------------------
            with (
                tc.tile_pool(name="pr", bufs=2, space="PSUM") as pr,
                tc.tile_pool(name="pb", bufs=4, space="PSUM") as pb,
            ):
                for h in range(2):
                    psum_row = pr.tile([B, 512], f32, tag="pr")
                    for k in range(KC):
                        nc.tensor.matmul(
                            psum_row[:],
                            vvt_t[:, ts(k, B)],
                            wp_t[k][:, ts(h, 512)],
                            start=(k == 0),
                            stop=False,
                        )
                    nc.tensor.matmul(
                        psum_row[:],
                        ones_b[0:1, :],
                        bp_t[0:1, ts(h, 512)],
                        start=False,
                        stop=True,
                    )
                    nc.vector.tensor_copy(row_t[0:B, ts(h, 512)], psum_row[:])

                    # bcast: bc[t, n] = row[b, n] for all t
                    for b in range(B):
                        psum_bc = pb.tile([128, 512], f32, tag="pb")
                        nc.tensor.matmul(
                            psum_bc[:],
                            sel_t[0:B, ts(b, 128)],
                            row_t[0:B, ts(h, 512)],
                            start=True,
                            stop=True,
                        )
                        nc.vector.tensor_copy(bc_t[h][:, ts(b, 512)], psum_bc[:])
                    nc.sync.dma_start(
                        out[:, :, ts(h, 512)],
                        bc_t[h][:].rearrange("p (b c) -> p b c", b=B),
                    )

    nc.compile()
    return nc


def _get_built():
    global _BUILT
    if _BUILT is None:
        _BUILT = build_nc()
    return _BUILT


def make_in_map(inputs):
    vf = np.ascontiguousarray(np.asarray(inputs["visual_features"], np.float32))
    return {
        "wv": np.ascontiguousarray(np.asarray(inputs["Wv"], np.float32)),
        "wp": np.ascontiguousarray(np.asarray(inputs["Wp"], np.float32)),
        "vft": np.ascontiguousarray(vf.T),
        "bv": np.ascontiguousarray(np.asarray(inputs["bv"], np.float32))[None, :],
        "bp": np.ascontiguousarray(np.asarray(inputs["bp"], np.float32))[None, :],
    }


def run(inputs, trace=False, **kw):
    from concourse.bass_utils import run_bass_kernel_spmd

    nc = _get_built()
    in_map = make_in_map(inputs)
    res = run_bass_kernel_spmd(
        nc,
        [dict(in_map) for _ in range(N_CORES)],
        core_ids=list(range(N_CORES)),
        trace=trace,
        **kw,
    )
    full = np.empty((B, T, C), np.float32)
    for i, r in enumerate(res.results):
        full[:, i * TSH : (i + 1) * TSH, :] = r["out"].transpose(1, 0, 2)
    return full, res


def kernel(**inputs) -> np.ndarray:
    full, _ = run(inputs, trace=False)
    return full


# revision 12
# speedup vs baseline: 1.5505x; 1.3099x over previous
"""Trainium2 Bass kernel for nn_CrossAttention_47502338294587.

Math: the reference cross-attention has a single KV position broadcast over
all T query positions.  Softmax over a row of identical logits is uniform,
so attention output == v for every query, and the whole module collapses to

    out[b, t, :] = (visual_features[b] @ Wv + bv) @ Wp + bp      (for all t)

independent of x / Wq / Wk.  The device computes the two projections and
broadcasts the per-batch row over the T axis; the host only re-assembles
shards (pure layout, no arithmetic).

Sharding: tensor-parallel over the output channel dim C — core i computes
and writes out[:, :, i*128:(i+1)*128] (it loads full Wv but only its column
shard of Wp / bp).  With C-sharding, a core's whole output shard is one
[128, B*128] tile replicated over the 8 t-chunks, so the T-broadcast is a
single selector matmul + replicated DMAs.

Per-core structure:
  mm1:   vv = vf @ Wv + bv          stationary vf^T chunks, moving Wv (N=512)
  tr:    vv^T chunks via PE transpose
  mm2:   row_sh = vv @ Wp[:,ci] + bp[ci]   (N=128)
  bcast: rhs4[k, b*128+c] = row_sh[k,c]*(k==b)  (DVE), then
         bc[t, (b,c)] = ones^T @ rhs4 (one matmul), DMA'd 8x over t-chunks
"""

import os
import sys

import numpy as np

for _p in ("/opt/trn_rl_repo",):
    if _p not in sys.path and os.path.isdir(_p):
        sys.path.insert(0, _p)

B, T, C = 4, 1024, 1024
N_CORES = 8
CSH = C // N_CORES  # 128, C-shard per core
KC = C // 128  # 8 contraction chunks

_BUILT = None


def build_nc():
    """Build + compile the Bass program (one NeuronCore's SPMD body)."""
    import concourse.mybir as mybir
    import concourse.tile as tile
    from concourse import bacc
    from concourse.bass import ts

    f32 = mybir.dt.float32
    nc = bacc.Bacc("TRN2", target_bir_lowering=False, debug=False)

    wv = nc.dram_tensor("wv", [C, C], f32, kind="ExternalInput")
    wp_sh = nc.dram_tensor("wp_sh", [C, CSH], f32, kind="ExternalInput")
    vft = nc.dram_tensor("vft", [C, B], f32, kind="ExternalInput")  # vf transposed
    bv = nc.dram_tensor("bv", [1, C], f32, kind="ExternalInput")
    bp_sh = nc.dram_tensor("bp_sh", [1, CSH], f32, kind="ExternalInput")
    # out[t, b, c_local]; host re-assembles full[b, t, ci] = out[t, b, :]
    out = nc.dram_tensor("out", [T, B, CSH], f32, kind="ExternalOutput")

    def band_select(ap, mult, width):
        """keep 1.0 inside the band 0 <= y - mult*k <= width-1, else 0."""
        nc.gpsimd.memset(ap, 1.0)
        nc.gpsimd.affine_select(
            out=ap, in_=ap, compare_op=mybir.AluOpType.is_ge, fill=0.0,
            base=0, pattern=[[1, ap.shape[-1]]], channel_multiplier=-mult,
        )
        nc.gpsimd.affine_select(
            out=ap, in_=ap, compare_op=mybir.AluOpType.is_ge, fill=0.0,
            base=width - 1, pattern=[[-1, ap.shape[-1]]], channel_multiplier=mult,
        )

    with tile.TileContext(nc) as tc:
        with tc.tile_pool(name="sb", bufs=1) as sb:
            # ---- SBUF tiles -------------------------------------------------
            wv_t = [sb.tile([128, C], f32, name=f"wv{k}", tag=f"wv{k}") for k in range(KC)]
            wp_t = sb.tile([128, KC, CSH], f32, tag="wp_t")  # Wp[:,ci] chunked
            vft_t = sb.tile([128, KC, B], f32, tag="vft")
            bv_t = sb.tile([1, C], f32, tag="bv")
            bp_t = sb.tile([1, CSH], f32, tag="bp")
            ones_b = sb.tile([1, B], f32, tag="ones_b")
            ones_bp = sb.tile([B, 128], f32, tag="ones_bp")
            # sel[k, b*128 + c] = (k == b)
            sel_t = sb.tile([B, B * 128], f32, tag="sel")
            ident_t = sb.tile([B, B], f32, tag="ident")
            vv_sb = sb.tile([B, C], f32, tag="vv_sb")
            vvt_t = sb.tile([128, KC * B], f32, tag="vvt")  # vv^T, chunk k at cols [4k,4k+4)
            row_t = sb.tile([B, CSH], f32, tag="row")
            rhs4_t = sb.tile([B, B * CSH], f32, tag="rhs4")
            bc_t = sb.tile([128, B * CSH], f32, tag="bc")

            nc.vector.memset(ones_b[:], 1.0)
            nc.vector.memset(ones_bp[:], 1.0)
            band_select(sel_t[:], 128, 128)
            band_select(ident_t[:], 1, 1)

            # ---- DMA in (small tensors first, then the big Wv) -------------
            nc.sync.dma_start(vft_t[:], vft.rearrange("(k p) b -> p k b", p=128))
            nc.sync.dma_start(bv_t[:], bv[:, :])
            nc.sync.dma_start(bp_t[:], bp_sh[:, :])
            nc.sync.dma_start(wp_t[:], wp_sh.rearrange("(k p) c -> p k c", p=128))
            for k in range(KC):
                nc.sync.dma_start(wv_t[k][:], wv[ts(k, 128), :])

            # ---- mm1: vv[b, n] = sum_k vf[b, k] Wv[k, n] + bv[n] ------------
            with tc.tile_pool(name="pv", bufs=2, space="PSUM") as pv:
                psum_vv = [pv.tile([B, 512], f32, name=f"pvv{h}", tag=f"pvv{h}") for h in range(2)]
                for k in range(KC):
                    for h in range(2):
                        nc.tensor.matmul(
                            psum_vv[h][:],
                            vft_t[:, k, :],
                            wv_t[k][:, ts(h, 512)],
                            start=(k == 0),
                            stop=False,
                        )
                for h in range(2):
                    nc.tensor.matmul(
                        psum_vv[h][:],
                        ones_b[0:1, :],
                        bv_t[0:1, ts(h, 512)],
                        start=False,
                        stop=True,
                    )
                for h in range(2):
                    nc.vector.tensor_copy(vv_sb[0:B, ts(h, 512)], psum_vv[h][:])

            # ---- transpose vv -> vv^T chunks [128, B] -----------------------
            with tc.tile_pool(name="pt", bufs=4, space="PSUM") as pt:
                for k in range(KC):
                    psum_vvt = pt.tile([128, B], f32, tag="pvt")
                    nc.tensor.transpose(
                        psum_vvt[:], vv_sb[0:B, ts(k, 128)], ident_t[0:B, 0:B]
                    )
                    nc.vector.tensor_copy(vvt_t[:, ts(k, B)], psum_vvt[:])

            # ---- mm2: row_sh[b, c] = sum_k vv[b, k] Wp[k, ci_c] + bp[ci_c] --
            with (
                tc.tile_pool(name="pr", bufs=1, space="PSUM") as pr,
                tc.tile_pool(name="pb", bufs=1, space="PSUM") as pb,
            ):
                psum_row = pr.tile([B, CSH], f32, tag="pr")
                for k in range(KC):
                    nc.tensor.matmul(
                        psum_row[:],
                        vvt_t[:, ts(k, B)],
                        wp_t[:, k, :],
                        start=(k == 0),
                        stop=False,
                    )
                nc.tensor.matmul(
                    psum_row[:],
                    ones_b[0:1, :],
                    bp_t[0:1, :],
                    start=False,
                    stop=True,
                )
                nc.vector.tensor_copy(row_t[0:B, :], psum_row[:])

                # ---- broadcast: one [128, B*CSH] tile == whole shard --------
                # rhs4[k, b*128+c] = row_sh[k, c] * (k == b)
                for b in range(B):
                    nc.vector.tensor_mul(
                        rhs4_t[0:B, ts(b, CSH)], row_t[0:B, :], sel_t[0:B, ts(b, 128)]
                    )
                psum_bc = pb.tile([128, B * CSH], f32, tag="pb")
                nc.tensor.matmul(
                    psum_bc[:], ones_bp[0:B, :], rhs4_t[0:B, :], start=True, stop=True
                )
                nc.vector.tensor_copy(bc_t[:], psum_bc[:])

                # every t-chunk of the shard is the same tile
                for tc8 in range(KC):
                    nc.sync.dma_start(
                        out[ts(tc8, 128), :, :],
                        bc_t[:].rearrange("p (b c) -> p b c", b=B),
                    )

    nc.compile()
    return nc


def _get_built():
    global _BUILT
    if _BUILT is None:
        _BUILT = build_nc()
    return _BUILT


def make_in_maps(inputs):
    vf = np.ascontiguousarray(np.asarray(inputs["visual_features"], np.float32))
    wv = np.ascontiguousarray(np.asarray(inputs["Wv"], np.float32))
    wp = np.ascontiguousarray(np.asarray(inputs["Wp"], np.float32))
    bv = np.ascontiguousarray(np.asarray(inputs["bv"], np.float32))[None, :]
    bp = np.ascontiguousarray(np.asarray(inputs["bp"], np.float32))
    vft = np.ascontiguousarray(vf.T)
    maps = []
    for i in range(N_CORES):
        ci = slice(i * CSH, (i + 1) * CSH)
        maps.append(
            {
                "wv": wv,
                "wp_sh": np.ascontiguousarray(wp[:, ci]),
                "vft": vft,
                "bv": bv,
                "bp_sh": np.ascontiguousarray(bp[ci])[None, :],
            }
        )
    return maps


def run(inputs, trace=False, **kw):
    from concourse.bass_utils import run_bass_kernel_spmd

    nc = _get_built()
    res = run_bass_kernel_spmd(
        nc,
        make_in_maps(inputs),
        core_ids=list(range(N_CORES)),
        trace=trace,
        **kw,
    )
    full = np.empty((B, T, C), np.float32)
    for i, r in enumerate(res.results):
        # r["out"]: [T, B, CSH] -> full[:, :, ci]
        full[:, :, i * CSH : (i + 1) * CSH] = r["out"].transpose(1, 0, 2)
    return full, res


def kernel(**inputs) -> np.ndarray:
    full, _ = run(inputs, trace=False)
    return full


# revision 13
# speedup vs baseline: 1.8064x; 1.1651x over previous
"""Trainium2 Bass kernel for nn_CrossAttention_47502338294587.

Math: the reference cross-attention has a single KV position broadcast over
all T query positions.  Softmax over a row of identical logits is uniform,
so attention output == v for every query, and the whole module collapses to

    out[b, t, :] = (visual_features[b] @ Wv + bv) @ Wp + bp      (for all t)

independent of x / Wq / Wk.  The device computes the two projections and
broadcasts the per-batch row over the T axis; the host only does input
layout prep and shard re-assembly (pure data movement, no arithmetic).

Sharding: tensor-parallel over the output channel dim C — core i computes
and writes out[:, :, i*128:(i+1)*128] (it loads full Wv but only its column
shard of Wp / bp).  With C-sharding, a core's whole output shard is one
[128, B*128] tile replicated over the 8 t-chunks, so the T-broadcast is a
single selector matmul + one replicated DMA.

Per-core structure:
  mm1:   vv = vf @ Wv          stationary vf^T chunks, moving Wv (N=512)
         + bv fused into the PSUM->SBUF copy (DVE tensor_add)
  tr:    vv^T chunks via PE transpose
  mm2:   row_sh = vv @ Wp[:,ci] (+ bp[ci] fused into copy)
  bcast: rhs4[k, b*128+c] = row_sh[k,c]*(k==b)  (DVE), then
         bc[t, (b,c)] = ones^T @ rhs4 (one matmul),
         one DMA with a step-0 replicated source writes all 8 t-chunks
"""

import os
import sys

import numpy as np

for _p in ("/opt/trn_rl_repo",):
    if _p not in sys.path and os.path.isdir(_p):
        sys.path.insert(0, _p)

B, T, C = 4, 1024, 1024
N_CORES = 8
CSH = C // N_CORES  # 128, C-shard per core
KC = C // 128  # 8 contraction chunks

_BUILT = None


def build_nc():
    """Build + compile the Bass program (one NeuronCore's SPMD body)."""
    import concourse.bass as bass
    import concourse.mybir as mybir
    import concourse.tile as tile
    from concourse import bacc
    from concourse.bass import ts

    f32 = mybir.dt.float32
    nc = bacc.Bacc("TRN2", target_bir_lowering=False, debug=False)

    wv = nc.dram_tensor("wv", [C, C], f32, kind="ExternalInput")
    # host pre-packs these into the exact SBUF layouts (pure layout prep):
    wp_p = nc.dram_tensor("wp_p", [128, KC * CSH], f32, kind="ExternalInput")
    vft_p = nc.dram_tensor("vft_p", [128, KC * B], f32, kind="ExternalInput")
    bv4 = nc.dram_tensor("bv4", [B, C], f32, kind="ExternalInput")
    bp4 = nc.dram_tensor("bp4", [B, CSH], f32, kind="ExternalInput")
    # out[t, b, c_local]; host re-assembles full[b, t, ci] = out[t, b, :]
    out = nc.dram_tensor("out", [T, B, CSH], f32, kind="ExternalOutput")

    def band_select(ap, mult, width):
        """keep 1.0 inside the band 0 <= y - mult*k <= width-1, else 0."""
        nc.gpsimd.memset(ap, 1.0)
        nc.gpsimd.affine_select(
            out=ap, in_=ap, compare_op=mybir.AluOpType.is_ge, fill=0.0,
            base=0, pattern=[[1, ap.shape[-1]]], channel_multiplier=-mult,
        )
        nc.gpsimd.affine_select(
            out=ap, in_=ap, compare_op=mybir.AluOpType.is_ge, fill=0.0,
            base=width - 1, pattern=[[-1, ap.shape[-1]]], channel_multiplier=mult,
        )

    with tile.TileContext(nc) as tc:
        with tc.tile_pool(name="sb", bufs=1) as sb:
            # ---- SBUF tiles -------------------------------------------------
            wv_t = [sb.tile([128, C], f32, name=f"wv{k}", tag=f"wv{k}") for k in range(KC)]
            wp_t = sb.tile([128, KC, CSH], f32, tag="wp_t")
            vft_t = sb.tile([128, KC, B], f32, tag="vft")
            bv4_t = sb.tile([B, C], f32, tag="bv4")
            bp4_t = sb.tile([B, CSH], f32, tag="bp4")
            ones_bp = sb.tile([B, 128], f32, tag="ones_bp")
            # sel[k, b*128 + c] = (k == b)
            sel_t = sb.tile([B, B * 128], f32, tag="sel")
            ident_t = sb.tile([B, B], f32, tag="ident")
            vv_sb = sb.tile([B, C], f32, tag="vv_sb")
            vvt_t = sb.tile([128, KC * B], f32, tag="vvt")
            row_t = sb.tile([B, CSH], f32, tag="row")
            rhs4_t = sb.tile([B, B * CSH], f32, tag="rhs4")
            bc_t = sb.tile([128, B * CSH], f32, tag="bc")

            nc.vector.memset(ones_bp[:], 1.0)
            band_select(sel_t[:], 128, 128)
            band_select(ident_t[:], 1, 1)

            # ---- DMA in (first mm1 dependency first) ------------------------
            nc.sync.dma_start(wv_t[0][:], wv[ts(0, 128), :])
            nc.sync.dma_start(vft_t[:], vft_p.rearrange("p (k b) -> p k b", b=B))
            nc.sync.dma_start(bv4_t[:], bv4[:, :])
            nc.sync.dma_start(bp4_t[:], bp4[:, :])
            nc.sync.dma_start(wp_t[:], wp_p.rearrange("p (k c) -> p k c", c=CSH))
            for k in range(1, KC):
                nc.sync.dma_start(wv_t[k][:], wv[ts(k, 128), :])

            # ---- mm1: vv[b, n] = sum_k vf[b, k] Wv[k, n]  (+bv via DVE) -----
            with tc.tile_pool(name="pv", bufs=2, space="PSUM") as pv:
                psum_vv = [pv.tile([B, 512], f32, name=f"pvv{h}", tag=f"pvv{h}") for h in range(2)]
                for k in range(KC):
                    for h in range(2):
                        nc.tensor.matmul(
                            psum_vv[h][:],
                            vft_t[:, k, :],
                            wv_t[k][:, ts(h, 512)],
                            start=(k == 0),
                            stop=(k == KC - 1),
                        )
                for h in range(2):
                    nc.vector.tensor_add(
                        vv_sb[0:B, ts(h, 512)], psum_vv[h][:], bv4_t[0:B, ts(h, 512)]
                    )

            # ---- transpose vv -> vv^T chunks [128, B] -----------------------
            with tc.tile_pool(name="pt", bufs=4, space="PSUM") as pt:
                for k in range(KC):
                    psum_vvt = pt.tile([128, B], f32, tag="pvt")
                    nc.tensor.transpose(
                        psum_vvt[:], vv_sb[0:B, ts(k, 128)], ident_t[0:B, 0:B]
                    )
                    nc.vector.tensor_copy(vvt_t[:, ts(k, B)], psum_vvt[:])

            # ---- mm2: row_sh = vv @ Wp[:,ci]  (+bp via DVE) -----------------
            with (
                tc.tile_pool(name="pr", bufs=1, space="PSUM") as pr,
                tc.tile_pool(name="pb", bufs=1, space="PSUM") as pb,
            ):
                psum_row = pr.tile([B, CSH], f32, tag="pr")
                for k in range(KC):
                    nc.tensor.matmul(
                        psum_row[:],
                        vvt_t[:, ts(k, B)],
                        wp_t[:, k, :],
                        start=(k == 0),
                        stop=(k == KC - 1),
                    )
                nc.vector.tensor_add(row_t[0:B, :], psum_row[:], bp4_t[0:B, :])

                # ---- broadcast: one [128, B*CSH] tile == whole shard --------
                for b in range(B):
                    nc.vector.tensor_mul(
                        rhs4_t[0:B, ts(b, CSH)], row_t[0:B, :], sel_t[0:B, ts(b, 128)]
                    )
                psum_bc = pb.tile([128, B * CSH], f32, tag="pb")
                nc.tensor.matmul(
                    psum_bc[:], ones_bp[0:B, :], rhs4_t[0:B, :], start=True, stop=True
                )
                nc.vector.tensor_copy(bc_t[:], psum_bc[:])

                # one DMA: step-0 replicated source writes all 8 t-chunks
                ap = bc_t[:]
                rep = bass.AP(
                    ap.tensor, ap.offset, [list(ap.ap[0]), [0, KC], list(ap.ap[1])]
                )
                nc.sync.dma_start(
                    out.rearrange("(q p) b c -> p q (b c)", p=128), rep
                )

    nc.compile()
    return nc


def _get_built():
    global _BUILT
    if _BUILT is None:
        _BUILT = build_nc()
    return _BUILT


def make_in_maps(inputs):
    vf = np.asarray(inputs["visual_features"], np.float32)
    wv = np.ascontiguousarray(np.asarray(inputs["Wv"], np.float32))
    wp = np.asarray(inputs["Wp"], np.float32)
    bv = np.asarray(inputs["bv"], np.float32)
    bp = np.asarray(inputs["bp"], np.float32)
    # vft_p[p, k*B + b] = vf[b, k*128 + p]
    vft_p = np.ascontiguousarray(
        vf.T.reshape(KC, 128, B).transpose(1, 0, 2).reshape(128, KC * B)
    )
    bv4 = np.ascontiguousarray(np.broadcast_to(bv[None, :], (B, C)))
    maps = []
    for i in range(N_CORES):
        ci = slice(i * CSH, (i + 1) * CSH)
        # wp_p[p, k*CSH + c] = Wp[k*128 + p, ci_c]
        wp_p = np.ascontiguousarray(
            wp[:, ci].reshape(KC, 128, CSH).transpose(1, 0, 2).reshape(128, KC * CSH)
        )
        bp4 = np.ascontiguousarray(np.broadcast_to(bp[ci][None, :], (B, CSH)))
        maps.append(
            {"wv": wv, "wp_p": wp_p, "vft_p": vft_p, "bv4": bv4, "bp4": bp4}
        )
    return maps


def run(inputs, trace=False, **kw):
    from concourse.bass_utils import run_bass_kernel_spmd

    nc = _get_built()
    res = run_bass_kernel_spmd(
        nc,
        make_in_maps(inputs),
        core_ids=list(range(N_CORES)),
        trace=trace,
        **kw,
    )
    full = np.empty((B, T, C), np.float32)
    for i, r in enumerate(res.results):
        full[:, :, i * CSH : (i + 1) * CSH] = r["out"].transpose(1, 0, 2)
    return full, res


def kernel(**inputs) -> np.ndarray:
    full, _ = run(inputs, trace=False)
    return full
